# revision 1
# baseline (speedup 1.0000x reference)
"""CatLayer Trainium2 kernel (pure fp32).

Math: out[i,j,b,:] = W @ leaky_relu(concat(x[i,b,:], x[j,b,:])) + bias
Since leaky_relu is elementwise over the concat:
    y  = leaky_relu(x)                    # (l, b, d)
    A  = y @ W[:, :d].T + bias            # (l, b, d)   "xi half"
    B  = y @ W[:, d:].T                   # (l, b, d)   "xj half"
    out[i,j,b,:] = A[i,b,:] + B[j,b,:]

Sharding: i-rows of the (l x l) pair grid over 8 cores (12 rows each).
Every core computes B for all j from full x; A only for its own i rows
(supplied per-core as the packed xiT input).

Inputs are packed host-side into SBUF layout so each is a fully
contiguous DMA (partition dim leading):
    xT   (128, KT*T):   xT[p, k*T + t]  = x[t, 128k+p]
    xiT  (128, KT*TI):  xiT[p, k*TI+ti] = x_own[ti, 128k+p]
    W_in (128, 8*D):    W_in[p, g*D+c]  = W.T[128g+p, c]   (g<4: W1, g>=4: W2)
    bias (1, D)
    out  (12*l*b, d)

Engines:
    ACT: Prelu(alpha=0.1) + B-path PSUM->SBUF drains
    PE : A/B matmuls (fp32), one-hot E-matmul for the 16->128 partition
         broadcast of A[i]
    DVE: tensor_add for all output tiles + A-path PSUM drains
    DMA: big contiguous transfers; out stores are one per (i, j-group)

The j-tiles are grouped with a small first group so the first out-DMA can
start as soon as the first B tile is drained.
"""

import numpy as np
from contextlib import ExitStack

import concourse.bacc as bacc
import concourse.mybir as mybir
from concourse import tile
from concourse.bass_utils import run_bass_kernel_spmd

F32 = mybir.dt.float32
AF = mybir.ActivationFunctionType

L, Bdim, D = 96, 16, 512
NCORES = 8
LPC = L // NCORES          # 12 i-rows per core
T = L * Bdim               # 1536 (j,b) rows
NT = T // 128              # 12 j-tiles
KT = D // 128              # 4 k-tiles
TI = LPC * Bdim            # 192 own (i,b) rows
NA = TI // 32              # 6 A-row groups of 32
NEG_SLOPE = 0.1


def build_nc(repeats: int = 1, group_sizes=(1, 1, 1, 1, 1, 2, 2, 3), gps_groups=()):
    """Build the per-core Bass program (identical on all cores)."""
    assert sum(group_sizes) == NT
    g_off = [0]
    for g in group_sizes:
        g_off.append(g_off[-1] + g)

    nc = bacc.Bacc("TRN2", target_bir_lowering=False, debug=False)

    xT = nc.dram_tensor("xT", (128, KT * T), F32, kind="ExternalInput")
    xiT = nc.dram_tensor("xiT", (128, KT * TI), F32, kind="ExternalInput")
    w_in = nc.dram_tensor("w_in", (128, 2 * KT * D), F32, kind="ExternalInput")
    bias = nc.dram_tensor("bias", (1, D), F32, kind="ExternalInput")
    out = nc.dram_tensor("out", (LPC * T, D), F32, kind="ExternalOutput")

    # One-hot E for the 16->128 partition broadcast of A rows, replicated
    # with period 32 down all 128 rows so any legal 32-aligned window has
    # identical content: ec[par][g, p] == 1 iff g % 32 == 16*par + p % 16
    ec_np = np.zeros((2, 128, 128), np.float32)
    for par in range(2):
        for g in range(128):
            for p in range(128):
                if g % 32 == 16 * par + (p % 16):
                    ec_np[par, g, p] = 1.0
    ec_dram = nc.inline_tensor(ec_np, "Ec")

    with tile.TileContext(nc) as tc, ExitStack() as ctx:
        persist = ctx.enter_context(tc.tile_pool(name="persist", bufs=1))
        stage = ctx.enter_context(tc.tile_pool(name="stage", bufs=2))
        psum = ctx.enter_context(tc.tile_pool(name="psum", bufs=6, space="PSUM"))
        outp = ctx.enter_context(tc.tile_pool(name="outp", bufs=3))

        # ---- small constants
        bias_sb = persist.tile([1, D], F32, tag="bias", name="bias_sb")
        nc.scalar.dma_start(bias_sb[:], bias[:])
        ones_sb = persist.tile([1, 128], F32, tag="ones", name="ones_sb")
        nc.vector.memset(ones_sb[:], 1.0)
        ec_all = persist.tile([128, 256], F32, tag="ec", name="ec_all")
        nc.scalar.dma_start(
            ec_all[:].rearrange("g (a p) -> g a p", a=2),
            ec_dram.ap().rearrange("a g p -> g a p"),
        )
        ec_sb = [ec_all[:, :128], ec_all[:, 128:]]
        w_sb = persist.tile([128, 2 * KT * D], F32, tag="w", name="w_sb")

        def w1s(k):
            return w_sb[:, k * D : (k + 1) * D]

        def w2s(k):
            return w_sb[:, (KT + k) * D : (KT + k + 1) * D]

        # ---- PE warm-up: HAM runs the PE at half clock until it has seen
        # ~3.4us of activity. Issue dummy matmuls (ones x ones) that depend
        # only on the memset so the array is at full clock when real matmuls
        # arrive with the first inputs.
        warm_ps = psum.tile([128, 64], F32, tag="eps", bufs=2, name="warm_ps")
        for _ in range(16):
            nc.tensor.matmul(
                warm_ps[:], ones_sb[:1, :], ones_sb[:1, :64],
                start=True, stop=True,
            )

        for rep in range(repeats):
            # ---- input DMAs: xiT (A path, small) first, then W1, then the
            # x slices with the W2 half interleaved. Each is contiguous.
            xi_st = stage.tile(
                [128, KT * TI], F32, tag="xi_st", bufs=1, name=f"xi_st_{rep}"
            )
            nc.sync.dma_start(xi_st[:], xiT[:])
            if rep == 0:
                nc.sync.dma_start(w_sb[:, : KT * D], w_in[:, : KT * D])
                nc.sync.dma_start(w_sb[:, KT * D :], w_in[:, KT * D :])
            x_st = stage.tile(
                [128, KT * T], F32, tag="x_st", bufs=1, name=f"x_st_{rep}"
            )
            # Every k slice is split at column 512: the first four B j-tiles
            # read only columns [0, 512) of each slice, so loading the
            # "a" halves first lets the first out-DMAs enter the sync FIFO
            # before the "b" halves (emitted after the first add group) --
            # otherwise the in-order HWDGE FIFO delays the first store
            # until the whole input stream has drained.
            XA = 512
            for k in range(KT):
                nc.sync.dma_start(
                    x_st[:, k * T : k * T + XA], xT[:, k * T : k * T + XA]
                )

            # ---- leaky relu, sliced per k-tile so B matmuls start per-slice
            yiT = persist.tile([128, KT * TI], F32, tag="yiT", name=f"yiT_{rep}")
            for k in range(KT):
                nc.scalar.activation(
                    yiT[:, k * TI : (k + 1) * TI],
                    xi_st[:, k * TI : (k + 1) * TI],
                    AF.Prelu,
                    alpha=NEG_SLOPE,
                )
            yT = persist.tile([128, KT * T], F32, tag="yT", name=f"yT_{rep}")
            for k in range(KT):
                nc.scalar.activation(
                    yT[:, k * T : k * T + XA],
                    x_st[:, k * T : k * T + XA],
                    AF.Prelu,
                    alpha=NEG_SLOPE,
                )

            def emit_xb():
                # the deferred b-halves: columns [512, T) of every k slice
                for k in range(KT):
                    nc.sync.dma_start(
                        x_st[:, k * T + XA : (k + 1) * T],
                        xT[:, k * T + XA : (k + 1) * T],
                    )
                for k in range(KT):
                    nc.scalar.activation(
                        yT[:, k * T + XA : (k + 1) * T],
                        x_st[:, k * T + XA : (k + 1) * T],
                        AF.Prelu,
                        alpha=NEG_SLOPE,
                    )

            # ---- A = leaky_relu(xi) @ W1.T + bias in three M-groups
            # (128, 32, 64 rows). The 32-row group re-covers rows 96..127 so
            # every E-matmul window can start at a legal base partition
            # (matmul operands must share base partition in {0, 32, 64}).
            a_parts = {}   # w -> (tile, offset)

            def emit_a(tag, rows, col0, windows):
                aps = psum.tile(
                    [rows, D], F32, tag="ps32", bufs=2,
                    padded_shape=[128, D], name=f"aps_{rep}_{tag}"
                )
                for k in range(KT):
                    nc.tensor.matmul(
                        aps[:],
                        yiT[:, k * TI + col0 : k * TI + col0 + rows],
                        w1s(k),
                        start=(k == 0),
                        stop=False,
                    )
                nc.tensor.matmul(
                    aps[:], ones_sb[:1, :rows], bias_sb[:1, :],
                    start=False, stop=True,
                )
                aw = persist.tile(
                    [rows, D], F32, tag=f"a_{tag}", name=f"a_{rep}_{tag}"
                )
                nc.vector.tensor_copy(aw[:], aps[:])
                for w, off in windows:
                    a_parts[w] = (aw, off)

            # Emission order = engine program order (engines run in-order),
            # so everything is emitted in expected-readiness order: a
            # "frontier" schedule where each producer is followed by the adds
            # it unlocks.
            out_v = out.rearrange("(i j p) c -> i p j c", i=LPC, p=128)
            abc = persist.tile([128, LPC * D], F32, tag="abc", name=f"abc_{rep}")
            n_grp = len(group_sizes)
            b_grp = [None] * n_grp

            def emit_bgroup(g):
                gsz = group_sizes[g]
                bg = persist.tile(
                    [128, gsz * D], F32, tag=f"b_grp{g}", name=f"b_grp{g}_{rep}"
                )
                for q in range(gsz):
                    jt = g_off[g] + q
                    bps = psum.tile(
                        [128, D], F32, tag="ps", bufs=4, name=f"bps_{rep}_{jt}"
                    )
                    for k in range(KT):
                        nc.tensor.matmul(
                            bps[:],
                            yT[:, k * T + 128 * jt : k * T + 128 * (jt + 1)],
                            w2s(k),
                            start=(k == 0),
                            stop=(k == KT - 1),
                        )
                    nc.scalar.activation(bg[:, q * D : (q + 1) * D], bps[:], AF.Copy)
                b_grp[g] = bg

            def emit_abc(il):
                w, par = divmod(il, 2)
                src, off = a_parts[w]
                eps = psum.tile(
                    [128, D], F32, tag="eps", bufs=2, name=f"eps_{rep}_{il}"
                )
                nc.tensor.matmul(
                    eps[:],
                    ec_sb[par][off : off + 32],
                    src[off : off + 32, :],
                    start=True,
                    stop=True,
                )
                # early slices drain on DVE (ACT is stuck behind the relus in
                # its in-order stream); later ones go to ACT which has slack.
                if il < 4:
                    nc.vector.tensor_copy(abc[:, il * D : (il + 1) * D], eps[:])
                else:
                    nc.scalar.activation(
                        abc[:, il * D : (il + 1) * D], eps[:], AF.Copy
                    )

            def emit_add(il, g):
                gsz = group_sizes[g]
                use_gps = False
                eng = nc.gpsimd if use_gps else nc.vector
                pool_tag = "og" if use_gps else "ot"
                ot = outp.tile(
                    [128, gsz * D], F32, tag=pool_tag, bufs=8,
                    name=f"ot_{rep}_{il}_{g}"
                )
                a_slice = abc[:, il * D : (il + 1) * D]
                eng.tensor_add(
                    ot[:].rearrange("p (j c) -> p j c", c=D),
                    b_grp[g][:].rearrange("p (j c) -> p j c", c=D),
                    a_slice.unsqueeze(1).broadcast_to((128, gsz, D)),
                )
                nc.sync.dma_start(
                    out_v[il, :, g_off[g] : g_off[g + 1], :],
                    ot[:].rearrange("p (j c) -> p j c", c=D),
                )

            ready_il = []
            ready_g = []

            def unlock_il(*ils):
                for il in ils:
                    emit_abc(il)
                for il in ils:
                    ready_il.append(il)
                    for g in ready_g:
                        emit_add(il, g)

            def unlock_g(g):
                emit_bgroup(g)
                ready_g.append(g)
                for il in ready_il:
                    emit_add(il, g)

            # windows: w0..2 live in the 128-row A group at offsets 0/32/64,
            # w3 in its own 32-row group, w4..5 in the 64-row group.
            a_specs = {
                0: ("g0", 128, 0, [(0, 0), (1, 32), (2, 64)]),
                3: ("g0b", 32, 96, [(3, 0)]),
                4: ("g1", 64, 128, [(4, 0), (5, 32)]),
            }
            pairs = [(2 * p, 2 * p + 1) for p in range(LPC // 2)]
            gi = 0
            xb_done = False
            for pi, pair in enumerate(pairs):
                if pi in a_specs:
                    emit_a(*a_specs[pi])
                unlock_il(*pair)
                while gi < n_grp and (gi + 1) * len(pairs) <= (pi + 1) * n_grp:
                    unlock_g(gi)
                    gi += 1
                    if not xb_done:
                        emit_xb()
                        xb_done = True
            while gi < n_grp:
                unlock_g(gi)
                gi += 1

    nc.compile()
    return nc


def _pack_kt(arr_t, nfree):
    """(D, nfree) k-major -> (128, KT*nfree) partition-packed SBUF layout."""
    return np.ascontiguousarray(
        arr_t.reshape(KT, 128, nfree).transpose(1, 0, 2).reshape(128, KT * nfree)
    )


def make_in_maps(x, W, bias):
    x = np.asarray(x, np.float32)
    W = np.asarray(W, np.float32)
    bias = np.asarray(bias, np.float32)
    xT = _pack_kt(np.ascontiguousarray(x.reshape(T, D).T), T)
    w_all = np.ascontiguousarray(
        np.ascontiguousarray(W.T)
        .reshape(2 * KT, 128, D)
        .transpose(1, 0, 2)
        .reshape(128, 2 * KT * D)
    )
    b2 = np.ascontiguousarray(bias.reshape(1, D))
    maps = []
    for r in range(NCORES):
        xi = _pack_kt(
            np.ascontiguousarray(x[r * LPC : (r + 1) * LPC].reshape(TI, D).T), TI
        )
        maps.append({"xT": xT, "xiT": xi, "w_in": w_all, "bias": b2})
    return maps


_NC_CACHE = {}


def get_nc(repeats=1, group_sizes=(1, 1, 1, 1, 1, 2, 2, 3), gps_groups=()):
    key = (repeats, tuple(group_sizes), tuple(gps_groups))
    if key not in _NC_CACHE:
        _NC_CACHE[key] = build_nc(repeats=repeats, group_sizes=group_sizes, gps_groups=gps_groups)
    return _NC_CACHE[key]


def kernel(x, W, bias, group_sizes=(1, 1, 1, 1, 1, 2, 2, 3), gps_groups=()):
    nc = get_nc(1, group_sizes, gps_groups)
    maps = make_in_maps(x, W, bias)
    res = run_bass_kernel_spmd(nc, maps, list(range(NCORES)))
    outs = [res.results[r]["out"] for r in range(NCORES)]
    return np.concatenate(outs, axis=0).reshape(L * L, Bdim, D)



# revision 2
# speedup vs baseline: 1.6689x; 1.6689x over previous
"""CatLayer Trainium2 kernel (fp16 datapath, fp32 PSUM accumulate).

Math: out[i,j,b,:] = W @ leaky_relu(concat(x[i,b,:], x[j,b,:])) + bias
Since leaky_relu is elementwise over the concat:
    y  = leaky_relu(x)                    # (l, b, d)
    A  = y @ W[:, :d].T + bias            # (l, b, d)   "xi half"
    B  = y @ W[:, d:].T                   # (l, b, d)   "xj half"
    out[i,j,b,:] = A[i,b,:] + B[j,b,:]

Sharding: i-rows of the (l x l) pair grid over 8 cores (12 rows each).
Every core computes B for all j from full x; A from its own 12 i-rows.

The whole on-device datapath is fp16 (the harness gate is rel<2e-2;
fp16 end-to-end costs ~2e-3): halves every DMA byte, gives 1 cyc/row
matmuls on PE and the 4x packed-2-byte DVE mode for the adds. PSUM
accumulation stays fp32.

Per-core input packing (host side, all fp16, partition dim leading):
    xT   (128, KT*T): xT[p, k*T + t] = x_rot[t, 128k+p] where x_rot is
         x.reshape(T, D) cyclically rotated by -r*TI rows, so each
         core's own 192 (i,b) rows sit at t in [0, TI). The A-path
         reads them as a column slice of yT; no separate xi input.
         Host un-rotates the output with np.roll after the gather.
    W_in (128, 8*D): W_in[p, g*D+c] = W.T[128g+p, c] (g<4: W1, g>=4: W2)
    bias (1, D)
    out  (12*l*b, d) fp16, host converts to fp32.

Engines:
    ACT: Prelu + B-path PSUM->SBUF drains + late abc drains
    PE : A/B matmuls (fp16), one-hot E-matmul for the 16->128 partition
         broadcast of A[i]
    DVE: tensor_add for all output tiles (4x packed mode) + A drains
    DMA: x + stores on the SP queue, W/ec/bias on the ACT queue
"""

import numpy as np
from contextlib import ExitStack

import concourse.bacc as bacc
import concourse.mybir as mybir
from concourse import tile
from concourse.bass_utils import run_bass_kernel_spmd

F32 = mybir.dt.float32
F16 = mybir.dt.float16
AF = mybir.ActivationFunctionType

L, Bdim, D = 96, 16, 512
NCORES = 8
LPC = L // NCORES          # 12 i-rows per core
T = L * Bdim               # 1536 (j,b) rows
NT = T // 128              # 12 j-tiles
KT = D // 128              # 4 k-tiles
TI = LPC * Bdim            # 192 own (i,b) rows
NEG_SLOPE = 0.1


def build_nc(repeats: int = 1, group_sizes=(1, 1, 2, 4, 4), warm=16):
    """Build the per-core Bass program (identical on all cores)."""
    assert sum(group_sizes) == NT
    g_off = [0]
    for g in group_sizes:
        g_off.append(g_off[-1] + g)

    nc = bacc.Bacc("TRN2", target_bir_lowering=False, debug=False)

    xT = nc.dram_tensor("xT", (128, KT * T), F16, kind="ExternalInput")
    w_in = nc.dram_tensor("w_in", (128, 2 * KT * D), F16, kind="ExternalInput")
    bias = nc.dram_tensor("bias", (1, D), F16, kind="ExternalInput")
    out = nc.dram_tensor("out", (LPC * T, D), F16, kind="ExternalOutput")

    # One-hot E for the 16->128 partition broadcast of A rows, replicated
    # with period 32 down all 128 rows so any legal 32-aligned window has
    # identical content: ec[par][g, p] == 1 iff g % 32 == 16*par + p % 16
    ec_np = np.zeros((2, 128, 128), np.float16)
    for par in range(2):
        for g in range(128):
            for p in range(128):
                if g % 32 == 16 * par + (p % 16):
                    ec_np[par, g, p] = 1.0
    ec_dram = nc.inline_tensor(ec_np, "Ec")

    with tile.TileContext(nc) as tc, ExitStack() as ctx:
        persist = ctx.enter_context(tc.tile_pool(name="persist", bufs=1))
        stage = ctx.enter_context(tc.tile_pool(name="stage", bufs=2))
        psum = ctx.enter_context(tc.tile_pool(name="psum", bufs=6, space="PSUM"))
        outp = ctx.enter_context(tc.tile_pool(name="outp", bufs=3))

        # ---- small constants
        bias_sb = persist.tile([1, D], F16, tag="bias", name="bias_sb")
        nc.scalar.dma_start(bias_sb[:], bias[:])
        ones_sb = persist.tile([1, 128], F16, tag="ones", name="ones_sb")
        nc.vector.memset(ones_sb[:], 1.0)
        ec_all = persist.tile([128, 256], F16, tag="ec", name="ec_all")
        nc.scalar.dma_start(
            ec_all[:].rearrange("g (a p) -> g a p", a=2),
            ec_dram.ap().rearrange("a g p -> g a p"),
        )
        ec_sb = [ec_all[:, :128], ec_all[:, 128:]]
        w_sb = persist.tile([128, 2 * KT * D], F16, tag="w", name="w_sb")

        def w1s(k):
            return w_sb[:, k * D : (k + 1) * D]

        def w2s(k):
            return w_sb[:, (KT + k) * D : (KT + k + 1) * D]

        # ---- PE warm-up: HAM runs the PE at half clock until it has seen
        # ~3.4us of activity. Issue dummy matmuls (ones x ones) that depend
        # only on the memset so the array is at full clock when real matmuls
        # arrive with the first inputs.
        warm_ps = psum.tile([128, 64], F32, tag="eps", bufs=2, name="warm_ps")
        for _ in range(warm):
            nc.tensor.matmul(
                warm_ps[:], ones_sb[:1, :], ones_sb[:1, :64],
                start=True, stop=True,
            )

        for rep in range(repeats):
            # ---- input DMAs. W halves go on the ACT queue so the SP queue
            # only carries x + output stores. Every k slice of x is split at
            # column 512: the first four B j-tiles read only columns
            # [0, 512), so loading the "a" halves first lets the first out
            # stores run before the "b" halves are even fetched.
            if rep == 0:
                nc.scalar.dma_start(w_sb[:, KT * D :], w_in[:, KT * D :])
                nc.scalar.dma_start(w_sb[:, : KT * D], w_in[:, : KT * D])
            x_st = stage.tile(
                [128, KT * T], F16, tag="x_st", bufs=1, name=f"x_st_{rep}"
            )
            XA = 512
            for k in range(KT):
                nc.sync.dma_start(
                    x_st[:, k * T : k * T + XA], xT[:, k * T : k * T + XA]
                )

            # ---- leaky relu, sliced per k-tile so B matmuls start per-slice
            yT = persist.tile([128, KT * T], F16, tag="yT", name=f"yT_{rep}")
            for k in range(KT):
                nc.scalar.activation(
                    yT[:, k * T : k * T + XA],
                    x_st[:, k * T : k * T + XA],
                    AF.Prelu,
                    alpha=NEG_SLOPE,
                )

            def emit_xb():
                # the deferred b-halves: columns [512, T) of every k slice
                for k in range(KT):
                    nc.sync.dma_start(
                        x_st[:, k * T + XA : (k + 1) * T],
                        xT[:, k * T + XA : (k + 1) * T],
                    )
                for k in range(KT):
                    nc.scalar.activation(
                        yT[:, k * T + XA : (k + 1) * T],
                        x_st[:, k * T + XA : (k + 1) * T],
                        AF.Prelu,
                        alpha=NEG_SLOPE,
                    )

            # ---- A = leaky_relu(x_own) @ W1.T + bias in three M-groups
            # (128, 32, 64 rows) sliced straight out of yT columns [0, TI).
            # The 32-row group re-covers rows 96..127 so every E-matmul
            # window starts at a legal base partition ({0, 32, 64}).
            a_parts = {}   # w -> (tile, offset)

            def emit_a(tag, rows, col0, windows):
                aps = psum.tile(
                    [rows, D], F32, tag="ps32", bufs=2,
                    padded_shape=[128, D], name=f"aps_{rep}_{tag}"
                )
                for k in range(KT):
                    nc.tensor.matmul(
                        aps[:],
                        yT[:, k * T + col0 : k * T + col0 + rows],
                        w1s(k),
                        start=(k == 0),
                        stop=False,
                    )
                nc.tensor.matmul(
                    aps[:], ones_sb[:1, :rows], bias_sb[:1, :],
                    start=False, stop=True,
                )
                aw = persist.tile(
                    [rows, D], F16, tag=f"a_{tag}", name=f"a_{rep}_{tag}"
                )
                nc.vector.tensor_copy(aw[:], aps[:])
                for w, off in windows:
                    a_parts[w] = (aw, off)

            # Emission order = engine program order (engines run in-order),
            # so everything is emitted in expected-readiness order: a
            # "frontier" schedule where each producer is followed by the adds
            # it unlocks.
            out_v = out.rearrange("(i j p) c -> i p j c", i=LPC, p=128)
            abc = persist.tile([128, LPC * D], F16, tag="abc", name=f"abc_{rep}")
            n_grp = len(group_sizes)
            b_grp = [None] * n_grp

            def emit_bgroup(g):
                gsz = group_sizes[g]
                bg = persist.tile(
                    [128, gsz * D], F16, tag=f"b_grp{g}", name=f"b_grp{g}_{rep}"
                )
                for q in range(gsz):
                    jt = g_off[g] + q
                    bps = psum.tile(
                        [128, D], F32, tag="ps", bufs=4, name=f"bps_{rep}_{jt}"
                    )
                    for k in range(KT):
                        nc.tensor.matmul(
                            bps[:],
                            yT[:, k * T + 128 * jt : k * T + 128 * (jt + 1)],
                            w2s(k),
                            start=(k == 0),
                            stop=(k == KT - 1),
                        )
                    nc.scalar.activation(bg[:, q * D : (q + 1) * D], bps[:], AF.Copy)
                b_grp[g] = bg

            def emit_abc(il):
                w, par = divmod(il, 2)
                src, off = a_parts[w]
                eps = psum.tile(
                    [128, D], F32, tag="eps", bufs=2, name=f"eps_{rep}_{il}"
                )
                nc.tensor.matmul(
                    eps[:],
                    ec_sb[par][off : off + 32],
                    src[off : off + 32, :],
                    start=True,
                    stop=True,
                )
                # early slices drain on DVE (ACT is stuck behind the relus in
                # its in-order stream); later ones go to ACT which has slack.
                if il < 4:
                    nc.vector.tensor_copy(abc[:, il * D : (il + 1) * D], eps[:])
                else:
                    nc.scalar.activation(
                        abc[:, il * D : (il + 1) * D], eps[:], AF.Copy
                    )

            def emit_add(il, g):
                gsz = group_sizes[g]
                ot = outp.tile(
                    [128, gsz * D], F16, tag="ot", bufs=8,
                    name=f"ot_{rep}_{il}_{g}"
                )
                a_slice = abc[:, il * D : (il + 1) * D]
                nc.vector.tensor_add(
                    ot[:].rearrange("p (j c) -> p j c", c=D),
                    b_grp[g][:].rearrange("p (j c) -> p j c", c=D),
                    a_slice.unsqueeze(1).broadcast_to((128, gsz, D)),
                )
                nc.sync.dma_start(
                    out_v[il, :, g_off[g] : g_off[g + 1], :],
                    ot[:].rearrange("p (j c) -> p j c", c=D),
                )

            ready_il = []
            ready_g = []

            def unlock_il(*ils):
                for il in ils:
                    emit_abc(il)
                for il in ils:
                    ready_il.append(il)
                    for g in ready_g:
                        emit_add(il, g)

            def unlock_g(g):
                emit_bgroup(g)
                ready_g.append(g)
                for il in ready_il:
                    emit_add(il, g)

            # windows: w0..2 live in the 128-row A group at offsets 0/32/64,
            # w3 in its own 32-row group, w4..5 in the 64-row group.
            a_specs = {
                0: ("g0", 128, 0, [(0, 0), (1, 32), (2, 64)]),
                3: ("g0b", 32, 96, [(3, 0)]),
                4: ("g1", 64, 128, [(4, 0), (5, 32)]),
            }
            pairs = [(2 * p, 2 * p + 1) for p in range(LPC // 2)]
            gi = 0
            xb_done = False
            for pi, pair in enumerate(pairs):
                if pi in a_specs:
                    emit_a(*a_specs[pi])
                unlock_il(*pair)
                while gi < n_grp and (gi + 1) * len(pairs) <= (pi + 1) * (n_grp + 1):
                    unlock_g(gi)
                    gi += 1
                    if not xb_done:
                        emit_xb()
                        xb_done = True
            while gi < n_grp:
                unlock_g(gi)
                gi += 1

    nc.compile()
    return nc


def _pack_kt(arr_t, nfree):
    """(D, nfree) k-major -> (128, KT*nfree) partition-packed SBUF layout."""
    return np.ascontiguousarray(
        arr_t.reshape(KT, 128, nfree).transpose(1, 0, 2).reshape(128, KT * nfree)
    )


def make_in_maps(x, W, bias):
    x = np.asarray(x, np.float16)
    W = np.asarray(W, np.float16)
    bias = np.asarray(bias, np.float16)
    x_flat = x.reshape(T, D)
    w_all = np.ascontiguousarray(
        np.ascontiguousarray(W.T)
        .reshape(2 * KT, 128, D)
        .transpose(1, 0, 2)
        .reshape(128, 2 * KT * D)
    )
    b2 = np.ascontiguousarray(bias.reshape(1, D))
    maps = []
    for r in range(NCORES):
        xr = np.roll(x_flat, -r * TI, axis=0)
        xTr = _pack_kt(np.ascontiguousarray(xr.T), T)
        maps.append({"xT": xTr, "w_in": w_all, "bias": b2})
    return maps


_NC_CACHE = {}


def get_nc(repeats=1, group_sizes=(1, 1, 2, 4, 4), warm=16):
    key = (repeats, tuple(group_sizes), warm)
    if key not in _NC_CACHE:
        _NC_CACHE[key] = build_nc(
            repeats=repeats, group_sizes=group_sizes, warm=warm
        )
    return _NC_CACHE[key]


def kernel(x, W, bias, group_sizes=(1, 1, 2, 4, 4), warm=16):
    nc = get_nc(1, group_sizes, warm)
    maps = make_in_maps(x, W, bias)
    res = run_bass_kernel_spmd(nc, maps, list(range(NCORES)))
    parts = []
    for r in range(NCORES):
        o = res.results[r]["out"].reshape(LPC, T, D)
        parts.append(np.roll(o, r * TI, axis=1))
    full = np.concatenate(parts, axis=0)          # (L, T, D)
    return full.reshape(L * L, Bdim, D).astype(np.float32)


# revision 3
# speedup vs baseline: 1.7979x; 1.0773x over previous
"""CatLayer Trainium2 kernel (fp16 datapath, fp32 PSUM accumulate).

Math: out[i,j,b,:] = W @ leaky_relu(concat(x[i,b,:], x[j,b,:])) + bias
Since leaky_relu is elementwise over the concat:
    y  = leaky_relu(x)                    # (l, b, d)
    A  = y @ W[:, :d].T + bias            # (l, b, d)   "xi half"
    B  = y @ W[:, d:].T                   # (l, b, d)   "xj half"
    out[i,j,b,:] = A[i,b,:] + B[j,b,:]

Sharding: i-rows of the (l x l) pair grid over 8 cores (12 rows each).
Every core computes B for all j from full x; A from its own 12 i-rows.

The whole on-device datapath is fp16 (the harness gate is rel<2e-2;
fp16 end-to-end costs ~1e-3): halves every DMA byte, gives 1 cyc/row
matmuls on PE and the packed-2-byte DVE mode for the adds. PSUM
accumulation stays fp32.

Per-core input packing (host side, all fp16, partition dim leading):
    xT   (128, KT*T): xT[p, k*T + t] = x_rot[t, 128k+p] where x_rot is
         x.reshape(T, D) cyclically rotated by -r*TI rows, so each
         core's own 192 (i,b) rows sit at t in [0, TI). The A-path
         reads them as a column slice of yT; no separate xi input.
         Host un-rotates the output with np.roll after the gather.
    W_in (128, 8*D): W_in[p, g*D+c] = W.T[128g+p, c] (g<4: W1, g>=4: W2)
    bias (1, D)
    out  (12*l*b, d) fp16, host converts to fp32.

Queues (the cost model serializes all DMA transfers on one device at
360 B/ns, with ~650 ns of SEQ+HWDGE descriptor-gen per DMA on the
issuing queue, so loads are coalesced into one strided DMA each and
spread across queues):
    SP  : x a-halves (1 DMA), x b-halves (1 DMA), all output stores
    Pool: W (1 DMA, SWDGE path), ec one-hot, bias
    ACT : Prelu + B-path PSUM drains + late abc drains (no DMAs)
    PE  : A/B matmuls, one-hot E-matmul broadcast of A[i], warm-up
    DVE : tensor_add for all output tiles + A drains + early abc drains
"""

import numpy as np
from contextlib import ExitStack

import concourse.bacc as bacc
import concourse.mybir as mybir
from concourse import tile
from concourse.bass_utils import run_bass_kernel_spmd

F32 = mybir.dt.float32
F16 = mybir.dt.float16
AF = mybir.ActivationFunctionType

L, Bdim, D = 96, 16, 512
NCORES = 8
LPC = L // NCORES          # 12 i-rows per core
T = L * Bdim               # 1536 (j,b) rows
NT = T // 128              # 12 j-tiles
KT = D // 128              # 4 k-tiles
TI = LPC * Bdim            # 192 own (i,b) rows
NEG_SLOPE = 0.1
XA = 512                   # split column: first 4 j-tiles live in [0, XA)


def build_nc(repeats: int = 1, group_sizes=(2, 2, 4, 4), warm=16):
    """Build the per-core Bass program (identical on all cores)."""
    assert sum(group_sizes) == NT
    g_off = [0]
    for g in group_sizes:
        g_off.append(g_off[-1] + g)

    nc = bacc.Bacc("TRN2", target_bir_lowering=False, debug=False)

    xT = nc.dram_tensor("xT", (128, KT * T), F16, kind="ExternalInput")
    w_in = nc.dram_tensor("w_in", (128, 2 * KT * D), F16, kind="ExternalInput")
    bias = nc.dram_tensor("bias", (1, D), F16, kind="ExternalInput")
    out = nc.dram_tensor("out", (LPC * T, D), F16, kind="ExternalOutput")

    # One-hot E for the 16->128 partition broadcast of A rows, replicated
    # with period 32 down all 128 partitions so any legal 32-aligned window
    # has identical content: ec[g, a*128 + p] == 1 iff g % 32 == 16*a + p % 16
    # (g = partition = contraction row, p = output partition, a = half).
    ec_np = np.zeros((128, 256), np.float16)
    for g in range(128):
        for a in range(2):
            for p in range(128):
                if g % 32 == 16 * a + (p % 16):
                    ec_np[g, a * 128 + p] = 1.0
    ec_dram = nc.inline_tensor(ec_np, "Ec")

    with tile.TileContext(nc) as tc, ExitStack() as ctx:
        persist = ctx.enter_context(tc.tile_pool(name="persist", bufs=1))
        stage = ctx.enter_context(tc.tile_pool(name="stage", bufs=2))
        psum = ctx.enter_context(tc.tile_pool(name="psum", bufs=6, space="PSUM"))
        outp = ctx.enter_context(tc.tile_pool(name="outp", bufs=3))

        # ---- constants + weights on the otherwise-idle Pool queue
        w_sb = persist.tile([128, 2 * KT * D], F16, tag="w", name="w_sb")
        nc.gpsimd.dma_start(w_sb[:], w_in[:])
        ec_all = persist.tile([128, 256], F16, tag="ec", name="ec_all")
        nc.gpsimd.dma_start(ec_all[:], ec_dram.ap())
        bias_sb = persist.tile([1, D], F16, tag="bias", name="bias_sb")
        nc.gpsimd.dma_start(bias_sb[:], bias[:])
        ones_sb = persist.tile([1, 128], F16, tag="ones", name="ones_sb")
        nc.vector.memset(ones_sb[:], 1.0)
        ec_sb = [ec_all[:, :128], ec_all[:, 128:]]

        def w1s(k):
            return w_sb[:, k * D : (k + 1) * D]

        def w2s(k):
            return w_sb[:, (KT + k) * D : (KT + k + 1) * D]

        # ---- PE warm-up: HAM ramps the PE clock only while it's busy.
        # Wide dummy matmuls keep it continuously busy (and fully ramped)
        # until the W load lands and real matmuls start.
        warm_ps = psum.tile([128, D], F32, tag="eps", bufs=2, name="warm_ps")
        for _ in range(warm):
            nc.tensor.matmul(
                warm_ps[:], ones_sb[:1, :], ones_sb[:1, :1].broadcast_to((1, D)),
                start=True, stop=True,
            )

        for rep in range(repeats):
            # ---- x loads: one strided DMA for the a-halves (columns
            # [0, XA) of every k slice), one for the b-halves.
            x_st = stage.tile(
                [128, KT * T], F16, tag="x_st", bufs=1, name=f"x_st_{rep}"
            )
            x_v = x_st[:].rearrange("p (k t) -> p k t", k=KT)
            xT_v = xT.ap().rearrange("p (k t) -> p k t", k=KT)
            nc.sync.dma_start(x_v[:, :, :XA], xT_v[:, :, :XA])

            # ---- leaky relu on the a-halves, one instruction
            yT = persist.tile([128, KT * T], F16, tag="yT", name=f"yT_{rep}")
            y_v = yT[:].rearrange("p (k t) -> p k t", k=KT)
            nc.scalar.activation(
                y_v[:, :, :XA], x_v[:, :, :XA], AF.Prelu, alpha=NEG_SLOPE
            )

            def emit_xb():
                nc.sync.dma_start(x_v[:, :, XA:], xT_v[:, :, XA:])
                nc.scalar.activation(
                    y_v[:, :, XA:], x_v[:, :, XA:], AF.Prelu, alpha=NEG_SLOPE
                )

            # ---- A = leaky_relu(x_own) @ W1.T + bias in three M-groups
            # (128, 32, 64 rows) sliced straight out of yT columns [0, TI).
            # The 32-row group re-covers rows 96..127 so every E-matmul
            # window starts at a legal base partition ({0, 32, 64}).
            a_parts = {}   # w -> (tile, offset)

            def emit_a(tag, rows, col0, windows):
                aps = psum.tile(
                    [rows, D], F32, tag="ps32", bufs=2,
                    padded_shape=[128, D], name=f"aps_{rep}_{tag}"
                )
                for k in range(KT):
                    nc.tensor.matmul(
                        aps[:],
                        yT[:, k * T + col0 : k * T + col0 + rows],
                        w1s(k),
                        start=(k == 0),
                        stop=False,
                    )
                nc.tensor.matmul(
                    aps[:], ones_sb[:1, :rows], bias_sb[:1, :],
                    start=False, stop=True,
                )
                aw = persist.tile(
                    [rows, D], F16, tag=f"a_{tag}", name=f"a_{rep}_{tag}"
                )
                nc.vector.tensor_copy(aw[:], aps[:])
                for w, off in windows:
                    a_parts[w] = (aw, off)

            # Emission order = engine program order (engines run in-order),
            # so everything is emitted in expected-readiness order: a
            # "frontier" schedule where each producer is followed by the adds
            # it unlocks.
            out_v = out.rearrange("(i j p) c -> i p j c", i=LPC, p=128)
            abc = persist.tile([128, LPC * D], F16, tag="abc", name=f"abc_{rep}")
            n_grp = len(group_sizes)
            b_grp = [None] * n_grp

            def emit_bgroup(g):
                gsz = group_sizes[g]
                bg = persist.tile(
                    [128, gsz * D], F16, tag=f"b_grp{g}", name=f"b_grp{g}_{rep}"
                )
                for q in range(gsz):
                    jt = g_off[g] + q
                    bps = psum.tile(
                        [128, D], F32, tag="ps", bufs=4, name=f"bps_{rep}_{jt}"
                    )
                    for k in range(KT):
                        nc.tensor.matmul(
                            bps[:],
                            yT[:, k * T + 128 * jt : k * T + 128 * (jt + 1)],
                            w2s(k),
                            start=(k == 0),
                            stop=(k == KT - 1),
                        )
                    nc.scalar.activation(bg[:, q * D : (q + 1) * D], bps[:], AF.Copy)
                b_grp[g] = bg

            def emit_abc(il):
                w, par = divmod(il, 2)
                src, off = a_parts[w]
                eps = psum.tile(
                    [128, D], F32, tag="eps", bufs=2, name=f"eps_{rep}_{il}"
                )
                nc.tensor.matmul(
                    eps[:],
                    ec_sb[par][off : off + 32],
                    src[off : off + 32, :],
                    start=True,
                    stop=True,
                )
                # early slices drain on DVE (ACT is stuck behind the relus in
                # its in-order stream); later ones go to ACT which has slack.
                if il < 4:
                    nc.vector.tensor_copy(abc[:, il * D : (il + 1) * D], eps[:])
                else:
                    nc.scalar.activation(
                        abc[:, il * D : (il + 1) * D], eps[:], AF.Copy
                    )

            def emit_add(il, g):
                gsz = group_sizes[g]
                ot = outp.tile(
                    [128, gsz * D], F16, tag="ot", bufs=8,
                    name=f"ot_{rep}_{il}_{g}"
                )
                a_slice = abc[:, il * D : (il + 1) * D]
                nc.vector.tensor_add(
                    ot[:].rearrange("p (j c) -> p j c", c=D),
                    b_grp[g][:].rearrange("p (j c) -> p j c", c=D),
                    a_slice.unsqueeze(1).broadcast_to((128, gsz, D)),
                )
                nc.sync.dma_start(
                    out_v[il, :, g_off[g] : g_off[g + 1], :],
                    ot[:].rearrange("p (j c) -> p j c", c=D),
                )

            ready_il = []
            ready_g = []

            def unlock_il(*ils):
                for il in ils:
                    emit_abc(il)
                for il in ils:
                    ready_il.append(il)
                    for g in ready_g:
                        emit_add(il, g)

            def unlock_g(g):
                emit_bgroup(g)
                ready_g.append(g)
                for il in ready_il:
                    emit_add(il, g)

            # windows: w0..2 live in the 128-row A group at offsets 0/32/64,
            # w3 in its own 32-row group, w4..5 in the 64-row group.
            a_specs = {
                0: ("g0", 128, 0, [(0, 0), (1, 32), (2, 64)]),
                3: ("g0b", 32, 96, [(3, 0)]),
                4: ("g1", 64, 128, [(4, 0), (5, 32)]),
            }
            pairs = [(2 * p, 2 * p + 1) for p in range(LPC // 2)]
            gi = 0
            xb_done = False
            for pi, pair in enumerate(pairs):
                if pi in a_specs:
                    emit_a(*a_specs[pi])
                unlock_il(*pair)
                while gi < n_grp and (gi + 1) * len(pairs) <= (pi + 1) * (n_grp + 1):
                    unlock_g(gi)
                    gi += 1
                    if not xb_done:
                        emit_xb()
                        xb_done = True
            while gi < n_grp:
                unlock_g(gi)
                gi += 1

    nc.compile()
    return nc


def _pack_kt(arr_t, nfree):
    """(D, nfree) k-major -> (128, KT*nfree) partition-packed SBUF layout."""
    return np.ascontiguousarray(
        arr_t.reshape(KT, 128, nfree).transpose(1, 0, 2).reshape(128, KT * nfree)
    )


def make_in_maps(x, W, bias):
    x = np.asarray(x, np.float16)
    W = np.asarray(W, np.float16)
    bias = np.asarray(bias, np.float16)
    x_flat = x.reshape(T, D)
    w_all = np.ascontiguousarray(
        np.ascontiguousarray(W.T)
        .reshape(2 * KT, 128, D)
        .transpose(1, 0, 2)
        .reshape(128, 2 * KT * D)
    )
    b2 = np.ascontiguousarray(bias.reshape(1, D))
    maps = []
    for r in range(NCORES):
        xr = np.roll(x_flat, -r * TI, axis=0)
        xTr = _pack_kt(np.ascontiguousarray(xr.T), T)
        maps.append({"xT": xTr, "w_in": w_all, "bias": b2})
    return maps


_NC_CACHE = {}


def get_nc(repeats=1, group_sizes=(2, 2, 4, 4), warm=16):
    key = (repeats, tuple(group_sizes), warm)
    if key not in _NC_CACHE:
        _NC_CACHE[key] = build_nc(
            repeats=repeats, group_sizes=group_sizes, warm=warm
        )
    return _NC_CACHE[key]


def kernel(x, W, bias, group_sizes=(2, 2, 4, 4), warm=16):
    nc = get_nc(1, group_sizes, warm)
    maps = make_in_maps(x, W, bias)
    res = run_bass_kernel_spmd(nc, maps, list(range(NCORES)))
    parts = []
    for r in range(NCORES):
        o = res.results[r]["out"].reshape(LPC, T, D)
        parts.append(np.roll(o, r * TI, axis=1))
    full = np.concatenate(parts, axis=0)          # (L, T, D)
    return full.reshape(L * L, Bdim, D).astype(np.float32)
